# revision 1
# baseline (speedup 1.0000x reference)
"""Self-contained GCN edge-dot kernel for 8 TRN2 NeuronCores.

kernel(**inputs) takes the FULL problem inputs and returns sigmoid edge
scores for every edge, computed SPMD across 8 cores with bass/bacc.

Strategy: nodes assigned to cores degree-balanced (edges sharded by dest
node); per-128-dest-block aggregation via one-hot selection matmuls
accumulated in PSUM; neighbor rows fetched with gpsimd dma_gather (int16
indices, tables split in lo/hi halves, double-buffered per half); the two
inter-layer tables (P2 = H1 @ W_pass2 bf16-padded, H2) exchanged with
AllGather collectives; final edge dot via expansion matmul + DVE
multiply/reduce; per-segment batched DVE edge-value scaling.
"""
import sys
sys.path.insert(0, "/opt/trn_rl_repo")
import numpy as np
import ml_dtypes
import concourse.bass as bass
import concourse.bacc as bacc
import concourse.mybir as mybir
from concourse import masks
from concourse.bass_utils import run_bass_kernel_spmd

F32 = mybir.dt.float32
BF16 = mybir.dt.bfloat16
I16 = mybir.dt.int16
AF = mybir.ActivationFunctionType
NCORES = 8


# ---------------------------------------------------------------- host planning
class Plan:
    pass


def plan_graph(edge_row, edge_col, edge_vals, n_nodes, blocks_per_core, cb):
    p = Plan()
    NB = blocks_per_core
    NPc = NB * 128
    NP = NPc * NCORES
    SPLIT = NP // 2
    assert SPLIT <= 32768 and n_nodes <= NP
    p.NB, p.NPc, p.NP, p.SPLIT, p.CB = NB, NPc, NP, SPLIT, cb

    E = len(edge_row)
    deg = np.bincount(edge_row, minlength=NP)
    order = np.argsort(-deg, kind="stable")
    nblocks = NCORES * NB
    newpos = np.empty(NP, np.int64)
    for g in range(nblocks):
        members = order[g::nblocks]
        c, b = g // NB, g % NB
        newpos[members] = c * NPc + b * 128 + np.arange(len(members))
    p.newpos = newpos
    perm = np.empty(NP, np.int64)
    perm[newpos] = np.arange(NP)
    p.perm = perm

    nr = newpos[edge_row]
    ns = newpos[edge_col]
    core = nr // NPc
    blk = (nr % NPc) // 128
    dloc = nr % 128
    half = (ns >= SPLIT).astype(np.int64)
    sidx = np.where(half == 0, ns, ns - SPLIT)

    buckets = {}
    for c in range(NCORES):
        m_c = core == c
        for b in range(NB):
            m_b = m_c & (blk == b)
            for h in (0, 1):
                buckets[(c, b, h)] = np.nonzero(m_b & (half == h))[0]
    G = np.zeros((NB, 2), np.int64)
    for b in range(NB):
        for h in (0, 1):
            mx = max(len(buckets[(c, b, h)]) for c in range(NCORES))
            G[b, h] = max(1 if h == 0 else 0, -(-mx // 128))
    p.G = G
    p.Gtot = int(G.sum())
    S = p.Gtot * 128

    p.chunks = [list(range(i, min(i + cb, NB))) for i in range(0, NB, cb)]
    segs = []
    gidx = 0
    for ci, cblocks in enumerate(p.chunks):
        for h in (0, 1):
            for b in cblocks:
                ng = int(G[b, h])
                segs.append((ci, h, b, gidx, ng))
                gidx += ng
    assert gidx == p.Gtot
    p.segs = segs
    p.Gmax_half = max(
        sum(int(G[b, h]) for b in cblocks) for cblocks in p.chunks for h in (0, 1)
    )

    p.idx16 = np.zeros((NCORES, S), np.int16)
    p.dloc = np.zeros((NCORES, S), np.float32)
    p.val = np.zeros((NCORES, S), np.float32)
    p.slot_of_edge = np.full(E, -1, np.int64)
    p.core_of_edge = core
    for c in range(NCORES):
        for (ci, h, b, g0, ng) in segs:
            e_ids = buckets[(c, b, h)]
            n = len(e_ids)
            assert n <= ng * 128
            sl = g0 * 128 + np.arange(n)
            p.idx16[c, sl] = sidx[e_ids]
            p.dloc[c, sl] = dloc[e_ids]
            p.val[c, sl] = edge_vals[e_ids]
            p.slot_of_edge[e_ids] = sl
    return p


def wrap_idx(idx_flat):
    S = len(idx_flat)
    w = idx_flat.reshape(S // 16, 16).T
    return np.tile(w, (8, 1)).copy()


def colmajor(a):
    S = len(a)
    return a.reshape(S // 128, 128).T.copy()


# ---------------------------------------------------------------- bass emission
class Counters:
    def __init__(self):
        self.val = {}
        self.last = {}

    def inc(self, sem, by):
        self.val[sem] = self.val.get(sem, 0) + by
        return self.val[sem]

    def cur(self, sem):
        return self.val.get(sem, 0)

    def wait(self, eng_ops, eng_name, sem, v):
        if v <= 0:
            return
        key = (eng_name, sem)
        if self.last.get(key, -1) >= v:
            return
        self.last[key] = v
        eng_ops.append(("wait", sem, v))


def build(plan):
    p = plan
    NB, NPc, NP, SPLIT, CB = p.NB, p.NPc, p.NP, p.SPLIT, p.CB
    Gtot, G, segs, chunks = p.Gtot, p.G, p.segs, p.chunks
    S = Gtot * 128
    DI, D1, D2 = 128, 128, 64
    B = 8

    nc = bacc.Bacc()
    dp = nc.declare_dram_parameter
    xg = dp("xg", [NP, DI], BF16, isOutput=False)
    xlT = dp("xlT", [128, NPc], F32, isOutput=False)
    idx_in = dp("idx16", [128, S // 16], I16, isOutput=False)
    dloc_in = dp("dloc", [128, Gtot], BF16, isOutput=False)
    val_in = dp("val", [128, Gtot], BF16, isOutput=False)
    w1p_in = dp("w1p", [DI, D1], F32, isOutput=False)
    w1s_in = dp("w1s", [DI, D1], F32, isOutput=False)
    w2p_in = dp("w2p", [D1, D2], F32, isOutput=False)
    w2s_in = dp("w2s", [D1, D2], F32, isOutput=False)
    b1_in = dp("b1", [D1, 1], F32, isOutput=False)
    b2_in = dp("b2rep", [128, D2], F32, isOutput=False)
    sx_out = dp("sx", [128, Gtot], F32, isOutput=True)

    p2_loc = nc.dram_tensor("p2_loc", [NPc, 128], BF16)
    p2_full = nc.dram_tensor("p2_full", [NP, 128], BF16, addr_space="Shared")
    h2_loc = nc.dram_tensor("h2_loc", [NPc, D2], F32)
    h2_full = nc.dram_tensor("h2_full", [NP, D2], F32, addr_space="Shared")

    GH = p.Gmax_half
    ops = {e: [] for e in ("sp", "pool", "dve", "act", "pe")}
    C = Counters()
    DMA, V, A, P, CC, PL = "dma", "v", "a", "p", "cc", "pl"
    GSH = (("g00", "g01"), ("g10", "g11"))
    ev = {}
    sp, pool, dve, act, pe = (ops[k] for k in ("sp", "pool", "dve", "act", "pe"))

    def seg_groups(ci, h):
        return [(b, g0, ng) for (c2, h2, b, g0, ng) in segs if c2 == ci and h2 == h]

    # ---- phase 0: loads
    _ld_names = ("idx", "dloc", "val", "xlT", "w1p", "w1s", "w2p", "w2s",
                 "b1", "b2")
    for name in _ld_names:
        sp.append(("dma_sb", name))
        C.inc(DMA, 16)
    for name in _ld_names:
        ev["ld_" + name] = (DMA, C.cur(DMA))
    pool.append(("iota",))
    pool.append(("ident",))
    ev["p0_pool"] = (PL, C.inc(PL, 1))

    batches = [(g0, min(B, Gtot - g0)) for g0 in range(0, Gtot, B)]
    batch_of_group = {}
    for bi, (g0, nb_) in enumerate(batches):
        for g in range(g0, g0 + nb_):
            batch_of_group[g] = bi

    p3_half = {}
    p3_gfirst = {}
    p3_slot = {}
    for (ci, h, b, g0, ng) in segs:
        sgs = seg_groups(ci, h)
        gf = sgs[0][1]
        for g in range(g0, g0 + ng):
            p3_half[g] = h
            p3_gfirst[g] = gf
            p3_slot[g] = ci % 2

    def emit_agg_phase(ph, Dg, scale, on_block_done):
        next_batch = [0]

        def ensure_onehots(up_to_group):
            while next_batch[0] < len(batches) and \
                    batches[next_batch[0]][0] <= up_to_group:
                bi = next_batch[0]
                g0, nb_ = batches[bi]
                if bi >= 2:
                    pg0, pnb = batches[bi - 2]
                    C.wait(dve, "dve", P, ev[f"{ph}_pe_g{pg0 + pnb - 1}"][1])
                C.wait(dve, "dve", DMA, ev["ld_dloc"][1])
                C.wait(dve, "dve", PL, ev["p0_pool"][1])
                dve.append(("onehot", bi, g0, nb_))
                ev[f"{ph}_oh_b{bi}"] = (V, C.inc(V, 1))
                next_batch[0] += 1

        for ci, cblocks in enumerate(chunks):
            for h in (0, 1):
                sgs = seg_groups(ci, h)
                gsum = sum(ng for (_, _, ng) in sgs)
                if gsum == 0:
                    continue
                g_first = sgs[0][1]
                # gather-buffer reuse: consumer of previous same-half chunk done
                prevs = []
                for cj in range(ci - 1, -1, -1):
                    sg2 = seg_groups(cj, h)
                    if sum(n for (_, _, n) in sg2):
                        prevs.append(sg2)
                        if len(prevs) == 2:
                            break
                prev = prevs[1] if len(prevs) == 2 else None
                if prev is not None:
                    lastg = prev[-1][1] + prev[-1][2] - 1
                    if ph == "p3":
                        C.wait(pool, "pool", V, ev[f"p3_mult_g{lastg}"][1])
                    else:
                        C.wait(pool, "pool", P, ev[f"{ph}_pe_g{lastg}"][1])
                C.wait(pool, "pool", DMA, ev["ld_idx"][1])
                pool.append(("gather", ph, ci, h, g_first, gsum, Dg))
                gs = GSH[h][ci % 2]
                ev[f"{ph}_gather_{ci}_{h}"] = (gs, C.inc(gs, 16))

                if scale:
                    C.wait(dve, "dve", GSH[h][ci % 2],
                           ev[f"{ph}_gather_{ci}_{h}"][1])
                    C.wait(dve, "dve", DMA, ev["ld_val"][1])
                    dve.append(("scaleb", ph, h, g_first, gsum, Dg, ci % 2))
                    ev[f"{ph}_scale_{ci}_{h}"] = (V, C.inc(V, 1))

                if ph != "p3":
                    C.wait(pe, "pe", GSH[h][ci % 2],
                           ev[f"{ph}_gather_{ci}_{h}"][1])
                    if scale:
                        C.wait(pe, "pe", V, ev[f"{ph}_scale_{ci}_{h}"][1])
                    for (b, g0, ng) in sgs:
                        for g in range(g0, g0 + ng):
                            ensure_onehots(g)
                            bi = batch_of_group[g]
                            C.wait(pe, "pe", V, ev[f"{ph}_oh_b{bi}"][1])
                            first = (h == 0) and (g == g0)
                            last = ((h == 1) and (g == g0 + ng - 1)) or \
                                   ((h == 0) and G[b, 1] == 0 and
                                    g == g0 + ng - 1)
                            if first:
                                # psum slot reuse by previous occupant's drain
                                pbev = (f"p1_aggcopy_b{b - CB}" if ph == "p1"
                                        else f"p2_h2add_b{b - CB}")
                                if pbev in ev:
                                    C.wait(pe, "pe", V, ev[pbev][1])
                            pe.append(("agg", ph, h, b, g, g_first, first,
                                       last, Dg, ci % 2))
                            ev[f"{ph}_pe_g{g}"] = (P, C.inc(P, 1))
                            if last:
                                ev[f"{ph}_agg_b{b}"] = (P, C.cur(P))
                                on_block_done(b)
                else:
                    C.wait(pe, "pe", GSH[h][ci % 2],
                           ev[f"{ph}_gather_{ci}_{h}"][1])
                    C.wait(pe, "pe", PL, ev["p0_pool"][1])
                    glist = [g for (b, g0, ng) in sgs for g in range(g0, g0 + ng)]
                    bmap = {g: b for (b, g0, ng) in sgs for g in range(g0, g0 + ng)}
                    for wstart in range(0, len(glist), 8):
                        window = glist[wstart:wstart + 8]
                        for g in window:
                            b = bmap[g]
                            ensure_onehots(g)
                            bi = batch_of_group[g]
                            C.wait(pe, "pe", V, ev[f"{ph}_oh_b{bi}"][1])
                            if f"p3_ocp_g{g - CB}" in ev:
                                C.wait(pe, "pe", A, ev[f"p3_ocp_g{g - CB}"][1])
                            pe.append(("p3_trans", g))
                            ev[f"p3_tr_g{g}"] = (P, C.inc(P, 1))
                            C.wait(act, "act", P, ev[f"p3_tr_g{g}"][1])
                            if f"p3_exp_g{g - CB}" in ev:
                                C.wait(act, "act", P, ev[f"p3_exp_g{g - CB}"][1])
                            act.append(("p3_ocp", g))
                            ev[f"p3_ocp_g{g}"] = (A, C.inc(A, 1))
                            C.wait(pe, "pe", A, ev[f"p3_ocp_g{g}"][1])
                            if f"p3_mult_g{g - 4}" in ev:
                                C.wait(pe, "pe", V, ev[f"p3_mult_g{g - 4}"][1])
                            pe.append(("p3_expand", g, b))
                            ev[f"p3_exp_g{g}"] = (P, C.inc(P, 1))
                            ev[f"{ph}_pe_g{g}"] = (P, C.cur(P))
                            C.wait(dve, "dve", P, ev[f"p3_exp_g{g}"][1])
                            dve.append(("p3_mult", g))
                            ev[f"p3_mult_g{g}"] = (V, C.inc(V, 1))
                        C.wait(dve, "dve", V, ev[f"p3_mult_g{window[-1]}"][1])
                        for g in window:
                            dve.append(("p3_red", g))
                            ev[f"p3_red_g{g}"] = (V, C.inc(V, 1))
                        C.wait(dve, "dve", V, ev[f"p3_red_g{window[-1]}"][1])

    # ================= PHASE 1 =================
    C.wait(pe, "pe", DMA, ev["ld_xlT"][1])

    def p1_block_done(b):
        C.wait(dve, "dve", P, ev[f"p1_agg_b{b}"][1])
        _p1_tail(b)
        _p2a_block(b)

    def _p1_tail(b):
        if f"p1_h1_b{b - 2}" in ev:
            C.wait(dve, "dve", P, ev[f"p1_h1_b{b - 2}"][1])
        dve.append(("aggcopy", b))
        ev[f"p1_aggcopy_b{b}"] = (V, C.inc(V, 1))
        C.wait(pe, "pe", V, ev[f"p1_aggcopy_b{b}"][1])
        if f"p1_relu_b{b - 1}" in ev:
            C.wait(pe, "pe", A, ev[f"p1_relu_b{b - 1}"][1])
        pe.append(("h1mm", b))
        ev[f"p1_h1_b{b}"] = (P, C.inc(P, 2))
        C.wait(act, "act", P, ev[f"p1_h1_b{b}"][1])
        C.wait(act, "act", DMA, ev["ld_b1"][1])
        act.append(("h1relu", b))
        ev[f"p1_relu_b{b}"] = (A, C.inc(A, 1))

    def _p2a_block(b):
        C.wait(pe, "pe", DMA, ev["ld_w2s"][1])
        C.wait(pe, "pe", A, ev[f"p1_relu_b{b}"][1])
        if f"p2a_p2cp_b{b - 1}" in ev:
            C.wait(pe, "pe", A, ev[f"p2a_p2cp_b{b - 1}"][1])
        pe.append(("p2mm", b))
        ev[f"p2a_mm_b{b}"] = (P, C.inc(P, 2))
        C.wait(act, "act", P, ev[f"p2a_mm_b{b}"][1])
        act.append(("p2cp", b))
        ev[f"p2a_p2cp_b{b}"] = (A, C.inc(A, 2))
        C.wait(sp, "sp", A, ev[f"p2a_p2cp_b{b}"][1])
        sp.append(("p2wr", b))
        ev[f"p2a_wr_b{b}"] = (DMA, C.inc(DMA, 16))

    emit_agg_phase("p1", DI, True, p1_block_done)

    # ================= PHASE 2a tail =================
    C.wait(dve, "dve", A, ev[f"p2a_p2cp_b{NB - 1}"][1])
    C.wait(dve, "dve", DMA, ev["ld_b2"][1])
    dve.append(("s2bias",))
    ev["p2a_s2bias"] = (V, C.inc(V, 1))

    # ================= PHASE 2b =================
    C.wait(pool, "pool", DMA, ev[f"p2a_wr_b{NB - 1}"][1])
    pool.append(("ag_p2",))
    ev["ag_p2"] = (CC, C.inc(CC, 1))
    C.wait(pool, "pool", CC, ev["ag_p2"][1])

    # ================= PHASE 2c =================
    def p2_block_done(b):
        C.wait(dve, "dve", P, ev[f"p2_agg_b{b}"][1])
        C.wait(dve, "dve", V, ev["p2a_s2bias"][1])
        if f"p2_relu_b{b - 2}" in ev:
            C.wait(dve, "dve", A, ev[f"p2_relu_b{b - 2}"][1])
        dve.append(("h2add", b))
        ev[f"p2_h2add_b{b}"] = (V, C.inc(V, 1))
        C.wait(act, "act", V, ev[f"p2_h2add_b{b}"][1])
        act.append(("h2relu", b))
        ev[f"p2_relu_b{b}"] = (A, C.inc(A, 1))
        C.wait(sp, "sp", A, ev[f"p2_relu_b{b}"][1])
        sp.append(("h2wr", b))
        ev[f"p2c_wr_b{b}"] = (DMA, C.inc(DMA, 16))

    C.wait(pe, "pe", V, ev[f"p1_aggcopy_b{NB - 1}"][1])
    emit_agg_phase("p2", DI, True, p2_block_done)

    # ================= PHASE 2d =================
    C.wait(pool, "pool", DMA, ev[f"p2c_wr_b{NB - 1}"][1])
    pool.append(("ag_h2",))
    ev["ag_h2"] = (CC, C.inc(CC, 1))
    C.wait(pool, "pool", CC, ev["ag_h2"][1])

    # ================= PHASE 3 =================
    C.wait(pe, "pe", V, ev[f"p2_h2add_b{NB - 1}"][1])
    C.wait(pe, "pe", A, ev[f"p2a_p2cp_b{NB - 1}"][1])
    emit_agg_phase("p3", D2, False, None)
    C.wait(act, "act", V, ev[f"p3_red_g{Gtot - 1}"][1])
    act.append(("sigmoid",))
    ev["sig"] = (A, C.inc(A, 1))
    C.wait(sp, "sp", A, ev["sig"][1])
    sp.append(("sxwr",))
    C.inc(DMA, 16)

    # ------------------------------------------------ emit to bass
    from contextlib import ExitStack
    _es = ExitStack()
    with _es:
        idx_sb = _es.enter_context(nc.sbuf_tensor("idx_sb", [128, S // 16], I16))
        dloc_sb = _es.enter_context(nc.sbuf_tensor("dloc_sb", [128, Gtot], BF16))
        val_sb = _es.enter_context(nc.sbuf_tensor("val_sb", [128, Gtot], BF16))
        xlT_sb = _es.enter_context(nc.sbuf_tensor("xlT_sb", [128, NPc], F32))
        w1p_sb = _es.enter_context(nc.sbuf_tensor("w1p_sb", [128, D1], F32))
        w1s_sb = _es.enter_context(nc.sbuf_tensor("w1s_sb", [128, D1], F32))
        w2p_sb = _es.enter_context(nc.sbuf_tensor("w2p_sb", [128, D2], F32))
        w2s_sb = _es.enter_context(nc.sbuf_tensor("w2s_sb", [128, D2], F32))
        b1_sb = _es.enter_context(nc.sbuf_tensor("b1_sb", [128, 1], F32))
        b2_sb = _es.enter_context(nc.sbuf_tensor("b2_sb", [128, D2], F32))
        iota_sb = _es.enter_context(nc.sbuf_tensor("iota_sb", [128, B, 128], BF16))
        ident_sb = _es.enter_context(nc.sbuf_tensor("ident_sb", [128, 128], BF16))
        oh_sb = _es.enter_context(nc.sbuf_tensor("oh_sb", [128, 2, B, 128], BF16))
        glo_sb = _es.enter_context(nc.sbuf_tensor("glo_sb", [128, 2, GH * DI], BF16))
        ghi_sb = _es.enter_context(nc.sbuf_tensor("ghi_sb", [128, 2, GH * DI], BF16))
        h1T_sb = _es.enter_context(nc.sbuf_tensor("h1T_sb", [128, NPc], F32))
        aggT_sb = _es.enter_context(nc.sbuf_tensor("aggT_sb", [128, 2, 128], F32))
        s2_sb = _es.enter_context(nc.sbuf_tensor("s2_sb", [128, NB, D2], F32))
        h2nm_sb = _es.enter_context(nc.sbuf_tensor("h2nm_sb", [128, NB, D2], F32))
        p2nm_sb = _es.enter_context(nc.sbuf_tensor("p2nm_sb", [128, NB, 128], BF16))
        h2pre_sb = _es.enter_context(nc.sbuf_tensor("h2pre_sb", [128, 2, D2], F32))
        osb_sb = _es.enter_context(nc.sbuf_tensor("osb_sb", [128, 4, 128], F32))
        prod_sb = _es.enter_context(nc.sbuf_tensor("prod_sb", [128, 8, D2], F32))
        dots_sb = _es.enter_context(nc.sbuf_tensor("dots_sb", [128, Gtot], F32))
        aggb = [_es.enter_context(nc.psum_tensor(f"aggb{k}", [128, 512], F32))
                for k in range(CB)]
        h1b = _es.enter_context(nc.psum_tensor("h1b", [128, 512], F32))
        p2b = _es.enter_context(nc.psum_tensor("p2b", [128, 512], F32))
        s2b = _es.enter_context(nc.psum_tensor("s2b", [128, 512], F32))
        r3b = _es.enter_context(nc.psum_tensor("r3b", [128, 512], F32))
        dma_s = _es.enter_context(nc.semaphore("dma_s"))
        g00_s = _es.enter_context(nc.semaphore("g00_s"))
        g01_s = _es.enter_context(nc.semaphore("g01_s"))
        g10_s = _es.enter_context(nc.semaphore("g10_s"))
        g11_s = _es.enter_context(nc.semaphore("g11_s"))
        v_s = _es.enter_context(nc.semaphore("v_s"))
        a_s = _es.enter_context(nc.semaphore("a_s"))
        p_s = _es.enter_context(nc.semaphore("p_s"))
        cc_s = _es.enter_context(nc.semaphore("cc_s"))
        pl_s = _es.enter_context(nc.semaphore("pl_s"))
        block = _es.enter_context(nc.Block())
        sems = {DMA: dma_s, "g00": g00_s, "g01": g01_s, "g10": g10_s,
                "g11": g11_s, V: v_s, A: a_s, P: p_s, CC: cc_s, PL: pl_s}

        def gv_half(h, Dg, ph, slot):
            buf = glo_sb if h == 0 else ghi_sb
            flat = buf[:, slot, :]
            if ph == "p3":
                flat = flat.bitcast(F32)
            return flat[:, : GH * Dg].rearrange("p (g f) -> p g f", f=Dg)

        sb_map = {"idx": idx_sb, "dloc": dloc_sb, "val": val_sb, "xlT": xlT_sb,
                  "w1p": w1p_sb, "w1s": w1s_sb, "w2p": w2p_sb, "w2s": w2s_sb,
                  "b1": b1_sb, "b2": b2_sb}
        in_map_t = {"idx": idx_in, "dloc": dloc_in, "val": val_in, "xlT": xlT,
                    "w1p": w1p_in, "w1s": w1s_in, "w2p": w2p_in, "w2s": w2s_in,
                    "b1": b1_in, "b2": b2_in}

        def oh_slot(g):
            bi = batch_of_group[g]
            return oh_sb[:, bi % 2, g - batches[bi][0], :], bi

        def run_ops(eng, name):
            for op in ops[name]:
                kind = op[0]
                if kind == "wait":
                    eng.wait_ge(sems[op[1]], op[2])
                elif kind == "dma_sb":
                    eng.dma_start(out=sb_map[op[1]][:], in_=in_map_t[op[1]][:]
                                  ).then_inc(dma_s, 16)
                elif kind == "iota":
                    eng.iota(iota_sb[:], pattern=[[0, B], [1, 128]], base=0,
                             channel_multiplier=0,
                             allow_small_or_imprecise_dtypes=True)
                    eng.drain()
                elif kind == "ident":
                    eng.memset(p2nm_sb[:], 0.0)
                    eng.drain()
                    eng.memset(ident_sb[:], 0.0)
                    eng.drain()
                    masks.make_identity(nc, ident_sb[:], nomemset=True)
                    eng.drain()
                    eng.memset(ident_sb[:1, :1], 1.0).then_inc(pl_s, 1)
                elif kind == "gather":
                    _, ph, ci, h, g_first, gsum, Dg = op
                    tbl = {"p1": xg, "p2": p2_full, "p3": h2_full}[ph]
                    half_tbl = tbl[:SPLIT, :] if h == 0 else tbl[SPLIT:, :]
                    gv = gv_half(h, Dg, ph, ci % 2)
                    eng.dma_gather(
                        gv[:, :gsum, :], half_tbl,
                        idx_sb[:, g_first * 8:(g_first + gsum) * 8],
                        num_idxs=gsum * 128, num_idxs_reg=gsum * 128,
                        elem_size=Dg, single_packet=False,
                    ).then_inc(sems[("g00", "g01", "g10", "g11")
                                    [h * 2 + ci % 2]], 16)
                elif kind == "onehot":
                    _, bi, g0, nb_ = op
                    eng.tensor_tensor(
                        out=oh_sb[:, bi % 2, :nb_, :],
                        in0=dloc_sb[:, g0:g0 + nb_, None].to_broadcast(
                            [128, nb_, 128]),
                        in1=iota_sb[:, :nb_, :],
                        op=mybir.AluOpType.is_equal,
                    ).then_inc(v_s, 1)
                elif kind == "scaleb":
                    _, ph, h, g_first, gsum, Dg, slot = op
                    Ds = 64 if ph == "p2" else Dg
                    sl = gv_half(h, Dg, ph, slot)[:, :gsum, :Ds]
                    eng.tensor_tensor(
                        out=sl, in0=sl,
                        in1=val_sb[:, g_first:g_first + gsum, None
                                   ].to_broadcast([128, gsum, Ds]),
                        op=mybir.AluOpType.mult).then_inc(v_s, 1)
                elif kind == "agg":
                    _, ph, h, b, g, g_first, first, last, Dg, slot = op
                    gv = gv_half(h, Dg, ph, slot)
                    ohs, _ = oh_slot(g)
                    if ph == "p1":
                        eng.matmul(aggb[b % CB][:, :128],
                                   lhsT=gv[:, g - g_first, :], rhs=ohs,
                                   start=first, stop=last).then_inc(p_s, 1)
                    else:
                        eng.matmul(aggb[b % CB][:, :128], lhsT=ohs,
                                   rhs=gv[:, g - g_first, :], start=first,
                                   stop=last).then_inc(p_s, 1)
                elif kind == "aggcopy":
                    b = op[1]
                    eng.tensor_copy(out=aggT_sb[:, b % 2, :],
                                    in_=aggb[b % CB][:, :128]).then_inc(v_s, 1)
                elif kind == "h1mm":
                    b = op[1]
                    eng.matmul(h1b[:, :128], lhsT=w1p_sb[:],
                               rhs=aggT_sb[:, b % 2, :], start=True,
                               stop=False).then_inc(p_s, 1)
                    eng.matmul(h1b[:, :128], lhsT=w1s_sb[:],
                               rhs=xlT_sb[:, b * 128:(b + 1) * 128],
                               start=False, stop=True).then_inc(p_s, 1)
                elif kind == "h1relu":
                    b = op[1]
                    eng.activation(h1T_sb[:, b * 128:(b + 1) * 128],
                                   h1b[:, :128], AF.Relu, bias=b1_sb[:]
                                   ).then_inc(a_s, 1)
                elif kind == "p2mm":
                    b = op[1]
                    eng.matmul(p2b[:, :D2],
                               lhsT=h1T_sb[:, b * 128:(b + 1) * 128],
                               rhs=w2p_sb[:], start=True, stop=True
                               ).then_inc(p_s, 1)
                    eng.matmul(s2b[:, :D2],
                               lhsT=h1T_sb[:, b * 128:(b + 1) * 128],
                               rhs=w2s_sb[:], start=True, stop=True
                               ).then_inc(p_s, 1)
                elif kind == "p2cp":
                    b = op[1]
                    eng.activation(p2nm_sb[:, b, :D2], p2b[:, :D2],
                                   AF.Copy).then_inc(a_s, 1)
                    eng.activation(s2_sb[:, b, :], s2b[:, :D2],
                                   AF.Copy).then_inc(a_s, 1)
                elif kind == "p2wr":
                    b = op[1]
                    eng.dma_start(out=p2_loc[b * 128:(b + 1) * 128, :],
                                  in_=p2nm_sb[:, b, :]).then_inc(dma_s, 16)
                elif kind == "s2bias":
                    eng.tensor_tensor(
                        out=s2_sb[:], in0=s2_sb[:],
                        in1=b2_sb[:, None, :].to_broadcast([128, NB, D2]),
                        op=mybir.AluOpType.add).then_inc(v_s, 1)
                elif kind == "ag_p2":
                    eng.collective_compute(
                        "AllGather", mybir.AluOpType.bypass,
                        replica_groups=[list(range(NCORES))],
                        ins=[p2_loc[:]], outs=[p2_full[:]],
                    ).then_inc(cc_s, 1)
                elif kind == "ag_h2":
                    eng.collective_compute(
                        "AllGather", mybir.AluOpType.bypass,
                        replica_groups=[list(range(NCORES))],
                        ins=[h2_loc[:]], outs=[h2_full[:]],
                    ).then_inc(cc_s, 1)
                elif kind == "h2add":
                    b = op[1]
                    eng.tensor_tensor(out=h2pre_sb[:, b % 2, :],
                                      in0=aggb[b % CB][:, :D2],
                                      in1=s2_sb[:, b, :],
                                      op=mybir.AluOpType.add).then_inc(v_s, 1)
                elif kind == "h2relu":
                    b = op[1]
                    eng.activation(h2nm_sb[:, b, :], h2pre_sb[:, b % 2, :],
                                   AF.Relu).then_inc(a_s, 1)
                elif kind == "h2wr":
                    b = op[1]
                    eng.dma_start(out=h2_loc[b * 128:(b + 1) * 128, :],
                                  in_=h2nm_sb[:, b, :]).then_inc(dma_s, 16)
                elif kind == "p3_trans":
                    g = op[1]
                    ohs, _ = oh_slot(g)
                    eng.transpose(out=aggb[g % CB][:].bitcast(BF16)[:, :128],
                                  in_=ohs,
                                  identity=ident_sb[:]).then_inc(p_s, 1)
                elif kind == "p3_ocp":
                    g = op[1]
                    eng.activation(osb_sb[:, g % CB, :],
                                   aggb[g % CB][:].bitcast(BF16)[:, :128],
                                   AF.Copy).then_inc(a_s, 1)
                elif kind == "p3_expand":
                    _, g, b = op
                    rb = (h1b, p2b, s2b, r3b)[g % 4]
                    eng.matmul(rb[:, :D2], lhsT=osb_sb[:, g % CB, :],
                               rhs=h2nm_sb[:, b, :], start=True, stop=True
                               ).then_inc(p_s, 1)
                elif kind == "p3_mult":
                    g = op[1]
                    gv = gv_half(p3_half[g], D2, "p3", p3_slot[g])
                    rb = (h1b, p2b, s2b, r3b)[g % 4]
                    eng.tensor_tensor(out=prod_sb[:, g % 8, :],
                                      in0=gv[:, g - p3_gfirst[g], :],
                                      in1=rb[:, :D2],
                                      op=mybir.AluOpType.mult).then_inc(v_s, 1)
                elif kind == "p3_red":
                    g = op[1]
                    eng.reduce_sum(out=dots_sb[:, g:g + 1],
                                   in_=prod_sb[:, g % 8, :],
                                   axis=mybir.AxisListType.X).then_inc(v_s, 1)
                elif kind == "sigmoid":
                    eng.activation(dots_sb[:], dots_sb[:], AF.Sigmoid
                                   ).then_inc(a_s, 1)
                elif kind == "sxwr":
                    eng.dma_start(out=sx_out[:], in_=dots_sb[:]
                                  ).then_inc(dma_s, 16)
                else:
                    raise ValueError(kind)

        @block.sync
        def _(e):
            run_ops(e, "sp")

        @block.gpsimd
        def _(e):
            run_ops(e, "pool")

        @block.vector
        def _(e):
            run_ops(e, "dve")

        @block.scalar
        def _(e):
            run_ops(e, "act")

        @block.tensor
        def _(e):
            run_ops(e, "pe")

    nc.compile()
    return nc


def host_prep(X, edge_row, edge_col, edge_vals, W1p, b1p, W1s, b1s,
              W2p, b2p, W2s, b2s, plan):
    p = plan
    NP, NPc = p.NP, p.NPc
    Xp = np.zeros((NP, X.shape[1]), np.float32)
    Xp[: X.shape[0]] = X
    Xgf = np.ascontiguousarray(Xp[p.perm])
    Xg = Xgf.astype(ml_dtypes.bfloat16)
    b1 = np.ascontiguousarray((b1p + b1s).astype(np.float32)[:, None])
    b2rep = np.ascontiguousarray(
        np.tile((b2p + b2s).astype(np.float32)[None, :], (128, 1)))
    in_maps = []
    for c in range(NCORES):
        in_maps.append({
            "xg": Xg, "xlT": np.ascontiguousarray(Xgf[c * NPc:(c + 1) * NPc].T),
            "idx16": wrap_idx(p.idx16[c]),
            "dloc": colmajor(p.dloc[c]).astype(ml_dtypes.bfloat16),
            "val": colmajor(p.val[c]).astype(ml_dtypes.bfloat16),
            "w1p": np.ascontiguousarray(W1p, np.float32),
            "w1s": np.ascontiguousarray(W1s, np.float32),
            "w2p": np.ascontiguousarray(W2p, np.float32),
            "w2s": np.ascontiguousarray(W2s, np.float32),
            "b1": b1, "b2rep": b2rep,
        })
    return in_maps


def unpermute_sx(results, plan, n_edges):
    p = plan
    sx = np.empty(n_edges, np.float32)
    for c in range(NCORES):
        flat = results[c]["sx"].T.reshape(-1)
        m = p.core_of_edge[:n_edges] == c
        sx[m] = flat[p.slot_of_edge[m]]
    return sx


_CACHE = {}


def kernel(X, edge_row, edge_col, edge_vals,
           W_pass1, b_pass1, W_self1, b_self1,
           W_pass2, b_pass2, W_self2, b_self2):
    X = np.asarray(X, np.float32)
    er = np.asarray(edge_row).astype(np.int64)
    ec = np.asarray(edge_col).astype(np.int64)
    ev_ = np.asarray(edge_vals, np.float32)
    n_nodes, n_edges = X.shape[0], len(er)

    key = (n_nodes, n_edges, int(er[0]), int(ec[0]))
    if key not in _CACHE:
        plan = plan_graph(er, ec, ev_, n_nodes, blocks_per_core=49, cb=4)
        nc = build(plan)
        _CACHE[key] = (plan, nc)
    plan, nc = _CACHE[key]

    in_maps = host_prep(X, er, ec, ev_,
                        np.asarray(W_pass1), np.asarray(b_pass1),
                        np.asarray(W_self1), np.asarray(b_self1),
                        np.asarray(W_pass2), np.asarray(b_pass2),
                        np.asarray(W_self2), np.asarray(b_self2), plan)
    res = run_bass_kernel_spmd(nc, in_maps, core_ids=list(range(NCORES)))
    return unpermute_sx(res.results, plan, n_edges)



# revision 12
# speedup vs baseline: 2.2056x; 2.2056x over previous
"""Self-contained GCN edge-dot kernel for 8 TRN2 NeuronCores (v2).

kernel(**inputs) takes the FULL problem inputs and returns sigmoid edge
scores for every edge, computed SPMD across 8 cores with bass/bacc.

Strategy vs v1: nodes assigned degree-balanced to (core, block) with a
piece-major gather-table numbering so the two inter-layer exchanges
(P2 = H1 @ W_pass2 and H2, both 64-wide bf16 packed two-nodes-per-256B
row) stream as per-piece AllGather collectives overlapped with compute;
one-hot selection matrices built per group on DVE/Pool via
tensor_scalar (is_equal x val, 4x DVE mode); phase-3 edge dot uses
PE transpose + batched ACT psum copy + batched DVE mult/reduce; edge
val folded into the one-hot so gathered rows are used unscaled.
"""
import sys
sys.path.insert(0, "/opt/trn_rl_repo")
import numpy as np
import ml_dtypes
import concourse.bass as bass
import concourse.bacc as bacc
import concourse.mybir as mybir
from concourse import masks
from concourse.bass_utils import run_bass_kernel_spmd

F32 = mybir.dt.float32
BF16 = mybir.dt.bfloat16
I16 = mybir.dt.int16
AF = mybir.ActivationFunctionType
ALU = mybir.AluOpType
NCORES = 8


# ---------------------------------------------------------------- host planning
class Plan:
    pass


def plan_graph(edge_row, edge_col, edge_vals, n_nodes, NB=49, CB=4,
               pstart=(0, 25)):
    p = Plan()
    NPc = NB * 128
    NP = NPc * NCORES
    assert n_nodes <= NP
    NPIECES = len(pstart)
    pstart = list(pstart) + [NB]
    pn = [pstart[k + 1] - pstart[k] for k in range(NPIECES)]
    p.NB, p.NPc, p.NP, p.CB, p.NPIECES = NB, NPc, NP, CB, NPIECES
    p.pn = pn

    E = len(edge_row)
    deg = np.bincount(edge_row, minlength=NP)
    order = np.argsort(-deg, kind="stable")
    nblocks = NCORES * NB
    newpos = np.empty(NP, np.int64)
    for g in range(nblocks):
        members = order[g::nblocks]
        c, b = g // NB, g % NB
        newpos[members] = c * NPc + b * 128 + np.arange(len(members))
    p.newpos = newpos
    perm = np.empty(NP, np.int64)
    perm[newpos] = np.arange(NP)
    p.perm = perm

    # piece-major gather-table row numbering: trow(c,b,l) for piece k
    #   = trowbase[k] + c*pn[k]*128 + (b - pstart[k])*128 + l
    trowbase = np.zeros(NPIECES + 1, np.int64)
    for k in range(NPIECES):
        trowbase[k + 1] = trowbase[k] + NCORES * pn[k] * 128
    p.trowbase = trowbase
    pos_c = newpos // NPc
    pos_b = (newpos % NPc) // 128
    pos_l = newpos % 128
    pn_arr = np.array(pn)
    pstart_arr = np.array(pstart[:NPIECES])
    pk = np.searchsorted(pstart_arr[1:] if NPIECES > 1 else np.array([NB]),
                         pos_b, side="right")
    pk = np.minimum(pk, NPIECES - 1)
    trow_of_node = (trowbase[pk] + pos_c * pn_arr[pk] * 128 +
                    (pos_b - pstart_arr[pk]) * 128 + pos_l)
    p.trow_of_node = trow_of_node  # original node id -> table row
    # inverse: table row -> original node id
    node_of_trow = np.empty(NP, np.int64)
    node_of_trow[trow_of_node] = np.arange(NP)
    p.node_of_trow = node_of_trow

    nr = newpos[edge_row]          # dest in (c,b,l) space
    core = nr // NPc
    blk = (nr % NPc) // 128
    dloc = nr % 128
    strow = trow_of_node[edge_col]  # source table row
    spiece = np.searchsorted(trowbase[1:NPIECES + 1], strow, side="right")
    sidx1 = strow - trowbase[spiece]            # idx within piece (p1, rows)
    sidx2 = sidx1 // 2                          # idx within piece (p2/p3 pairs)
    par = (strow % 2).astype(np.int64)          # parity within pair row

    # bucket (core, block, piece) -> even/odd edge lists
    G = np.zeros((NB, NPIECES), np.int64)
    buckets = {}
    for c in range(NCORES):
        m_c = core == c
        for b in range(NB):
            m_b = m_c & (blk == b)
            for k in range(NPIECES):
                m = m_b & (spiece == k)
                ev_ = np.nonzero(m & (par == 0))[0]
                od_ = np.nonzero(m & (par == 1))[0]
                buckets[(c, b, k)] = (ev_, od_)
                ng = max(-(-len(ev_) // 64), -(-len(od_) // 64))
                G[b, k] = max(G[b, k], ng)
    for b in range(NB):
        if G[b].sum() == 0:
            G[b, 0] = 1
    p.G = G
    p.Gtot = int(G.sum())
    S = p.Gtot * 128

    chunks = [list(range(i, min(i + CB, NB))) for i in range(0, NB, CB)]
    p.chunks = chunks
    segs = []
    gidx = 0
    firstg = {}
    lastg = {}
    for ci, cblocks in enumerate(chunks):
        for k in range(NPIECES):
            for b in cblocks:
                ng = int(G[b, k])
                if ng == 0:
                    continue
                segs.append((ci, k, b, gidx, ng))
                if b not in firstg:
                    firstg[b] = gidx
                lastg[b] = gidx + ng - 1
                gidx += ng
    assert gidx == p.Gtot
    p.segs = segs
    p.firstg, p.lastg = firstg, lastg
    # gather segments: one dma_gather per (ci, k) covering its blocks' groups
    gsegs = []
    i = 0
    while i < len(segs):
        ci, k, b, g0, ng = segs[i]
        j = i
        tot = 0
        while j < len(segs) and segs[j][0] == ci and segs[j][1] == k:
            tot += segs[j][4]
            j += 1
        gsegs.append((ci, k, g0, tot))
        i = j
    p.gsegs = gsegs
    p.GH = max(t for (_, _, _, t) in gsegs)
    p.block_of_g = {}
    for (ci, k, b, g0, ng) in segs:
        for g in range(g0, g0 + ng):
            p.block_of_g[g] = b

    p.idx1 = np.zeros((NCORES, S), np.int16)
    p.idx2 = np.zeros((NCORES, S), np.int16)
    p.dloc = np.zeros((NCORES, S), np.float32)
    p.val = np.zeros((NCORES, S), np.float32)
    p.slot_of_edge = np.full(E, -1, np.int64)
    p.core_of_edge = core
    for c in range(NCORES):
        for (ci, k, b, g0, ng) in segs:
            ev_, od_ = buckets[(c, b, k)]
            for half, eids in ((0, ev_), (1, od_)):
                n = len(eids)
                assert n <= ng * 64
                gi = np.arange(n) // 64
                sl = (g0 + gi) * 128 + half * 64 + np.arange(n) % 64
                p.idx1[c, sl] = sidx1[eids]
                p.idx2[c, sl] = sidx2[eids]
                p.dloc[c, sl] = dloc[eids]
                p.val[c, sl] = edge_vals[eids]
                p.slot_of_edge[eids] = sl
    return p


def wrap_idx(idx_flat):
    S = len(idx_flat)
    w = idx_flat.reshape(S // 16, 16).T
    return np.tile(w, (8, 1)).copy()


def colmajor(a):
    S = len(a)
    return a.reshape(S // 128, 128).T.copy()


# ---------------------------------------------------------------- emission fw
class Counters:
    def __init__(self):
        self.val = {}
        self.last = {}

    def inc(self, sem, by):
        self.val[sem] = self.val.get(sem, 0) + by
        return self.val[sem]

    def cur(self, sem):
        return self.val.get(sem, 0)

    def wait(self, eng_ops, eng_name, sem, v):
        if v <= 0:
            return
        key = (eng_name, sem)
        if self.last.get(key, -1) >= v:
            return
        self.last[key] = v
        eng_ops.append(("wait", sem, v))


DMA, V, A, P, PL, CC = "dma", "v", "a", "p", "pl", "cc"
GTS = ("gt0", "gt1", "gt2", "gt3")
WRS = ("wr0", "wr1")


def build(plan):
    p = plan
    NB, NPc, NP, CB = p.NB, p.NPc, p.NP, p.CB
    NPIECES, pn, G = p.NPIECES, p.pn, p.G
    Gtot, segs, gsegs, chunks = p.Gtot, p.segs, p.gsegs, p.chunks
    firstg, lastg, block_of_g = p.firstg, p.lastg, p.block_of_g
    S = Gtot * 128
    GH = p.GH
    NSLOT = 4      # gather buffer slots
    OHS = 16       # one-hot slots

    nc = bacc.Bacc()
    dp = nc.declare_dram_parameter
    xg = dp("xg", [NP, 128], BF16, isOutput=False)
    xlT_in = dp("xlT", [128, NPc], BF16, isOutput=False)
    idx1_in = dp("idx1", [128, S // 16], I16, isOutput=False)
    idx2_in = dp("idx2", [128, S // 16], I16, isOutput=False)
    dloc_in = dp("dloc", [128, Gtot], F32, isOutput=False)
    val_in = dp("val", [128, Gtot], F32, isOutput=False)
    w1p_in = dp("w1p", [128, 128], BF16, isOutput=False)
    w1s_in = dp("w1s", [128, 128], BF16, isOutput=False)
    w2p_in = dp("w2p", [128, 64], BF16, isOutput=False)
    w2s_in = dp("w2s", [128, 64], BF16, isOutput=False)
    b1_in = dp("b1", [128, 1], F32, isOutput=False)
    b2_in = dp("b2rep", [128, 64], F32, isOutput=False)
    sx_out = dp("sx", [128, Gtot], F32, isOutput=True)

    p2_loc = nc.dram_tensor("p2_loc", [NPc, 64], BF16)
    h2_loc = nc.dram_tensor("h2_loc", [NPc, 64], BF16)
    p2t = nc.dram_tensor("p2t", [NP // 2, 128], BF16, addr_space="Shared")
    h2t = nc.dram_tensor("h2t", [NP // 2, 128], BF16, addr_space="Shared")

    # piece boundaries in table spaces
    trowbase = p.trowbase
    pairbase = [int(trowbase[k] // 2) for k in range(NPIECES + 1)]
    locbase = []   # local p2_loc/h2_loc row range per piece
    acc = 0
    for k in range(NPIECES):
        locbase.append(acc)
        acc += pn[k] * 128

    ops = {e: [] for e in ("sp", "pool", "dve", "act", "pe")}
    C = Counters()
    ev = {}
    sp, pool, dve, act, pe = (ops[k] for k in ("sp", "pool", "dve", "act", "pe"))

    # ---- loads
    _ld_names = ("idx1", "idx2", "dloc", "val", "xlT", "w1p", "w1s", "w2p",
                 "w2s", "b1", "b2")
    for name in _ld_names:
        sp.append(("dma_sb", name))
        C.inc(DMA, 16)
    for name in _ld_names:
        ev["ld_" + name] = (DMA, C.cur(DMA))
    pool.append(("iota",))
    pool.append(("ident",))
    ev["p0_pool"] = (PL, C.inc(PL, 1))

    # last p1/p2 block using psum bank j (for bank-reuse waits)
    last_user = {}
    for b in range(NB):
        last_user[b % CB] = b
    pstart_list = [0]
    for k in range(NPIECES - 1):
        pstart_list.append(pstart_list[-1] + pn[k])
    piece_of_block = {}
    for b in range(NB):
        piece_of_block[b] = max(k for k in range(NPIECES)
                                if pstart_list[k] <= b)

    # ---------------- gather emission helper
    gseq = [0]

    def emit_gather(ph, ci, k, g0, gsum):
        s = gseq[0]
        slot = s % NSLOT
        # slot reuse: wait consumers of gather s-NSLOT
        prev = s - NSLOT
        if prev >= 0:
            pph, pg0, pgsum = gmeta[prev]
            if pph == "p3":
                C.wait(pool, "pool", V, ev[f"p3_multdone_g{pg0 + pgsum - 1}"][1])
            else:
                C.wait(pool, "pool", P, ev[f"{pph}_agg_g{pg0 + pgsum - 1}"][1])
        C.wait(pool, "pool", DMA, ev["ld_idx1" if ph == "p1" else "ld_idx2"][1])
        if ph == "p2":
            C.wait(pool, "pool", CC, k + 1)
        elif ph == "p3":
            C.wait(pool, "pool", CC, NPIECES + k + 1)
        pool.append(("gather", ph, k, g0, gsum, slot))
        gs = GTS[slot]
        ev[f"{ph}_gather_g{g0}"] = (gs, C.inc(gs, 16))
        gmeta.append((ph, g0, gsum))
        gseq[0] += 1
        return slot

    gmeta = []

    # ---------------- p1 / p2 aggregation phases
    def emit_agg_phase(ph):
        for (ci, k, g0s, gsum) in gsegs:
            slot = emit_gather(ph, ci, k, g0s, gsum)
            segs_here = [(b, g0, ng) for (ci2, k2, b, g0, ng) in segs
                         if ci2 == ci and k2 == k]
            for (b, g0, ng) in segs_here:
                for g in range(g0, g0 + ng):
                    # DVE: weighted one-hot build
                    C.wait(dve, "dve", DMA, ev["ld_val"][1])
                    C.wait(dve, "dve", PL, ev["p0_pool"][1])
                    if ph == "p1":
                        if g - OHS >= 0:
                            C.wait(dve, "dve", P, ev[f"p1_agg_g{g - OHS}"][1])
                    else:
                        # slot last used by p1's group lu (same residue)
                        lu = g + OHS * ((Gtot - 1 - g) // OHS)
                        C.wait(dve, "dve", P, ev[f"p1_agg_g{lu}"][1])
                        if g - OHS >= 0:
                            C.wait(dve, "dve", P, ev[f"p2_agg_g{g - OHS}"][1])
                    dve.append(("ohw", g))
                    ev[f"{ph}_oh_g{g}"] = (V, C.inc(V, 1))

                    # PE: aggregation matmul(s)
                    _gs, _gv = ev[f"{ph}_gather_g{g0s}"]
                    C.wait(pe, "pe", _gs, _gv)
                    C.wait(pe, "pe", V, ev[f"{ph}_oh_g{g}"][1])
                    first = g == firstg[b]
                    last = g == lastg[b]
                    if first:
                        # psum bank reuse
                        if ph == "p1":
                            if b - CB >= 0:
                                C.wait(pe, "pe", A, ev[f"p1_aggcopy_b{b - CB}"][1])
                        else:
                            lb = last_user[b % CB]
                            C.wait(pe, "pe", A, ev[f"p1_aggcopy_b{lb}"][1])
                            if b - CB >= 0:
                                C.wait(pe, "pe", V, ev[f"p2_h2add_b{b - CB}"][1])
                    j = g - g0s
                    if ph == "p1":
                        pe.append(("agg1", b, g, j, slot, first, last))
                        ev[f"p1_agg_g{g}"] = (P, C.inc(P, 1))
                    else:
                        pe.append(("agg2", b, g, j, slot, first, last))
                        ev[f"p2_agg_g{g}"] = (P, C.inc(P, 2))
                    if last:
                        ev[f"{ph}_aggstop_b{b}"] = (P, C.cur(P))
                        if ph == "p1":
                            p1_block_tail(b)
                        else:
                            p2_block_tail(b)

    def p1_block_tail(b):
        # ACT: aggcopy psum->sbuf bf16
        C.wait(act, "act", P, ev[f"p1_aggstop_b{b}"][1])
        if b - 2 >= 0:
            C.wait(act, "act", P, ev[f"p1_h1mm_b{b - 2}"][1])
        act.append(("aggcopy", b))
        ev[f"p1_aggcopy_b{b}"] = (A, C.inc(A, 1))
        # PE: h1 = relu(W1p^T aggT + W1s^T xlT + b1)
        C.wait(pe, "pe", DMA, ev["ld_xlT"][1])
        C.wait(pe, "pe", A, ev[f"p1_aggcopy_b{b}"][1])
        if b - 1 >= 0:
            C.wait(pe, "pe", A, ev[f"p1_h1relu_b{b - 1}"][1])
        pe.append(("h1mm", b))
        ev[f"p1_h1mm_b{b}"] = (P, C.inc(P, 2))
        C.wait(act, "act", P, ev[f"p1_h1mm_b{b}"][1])
        C.wait(act, "act", DMA, ev["ld_b1"][1])
        act.append(("h1relu", b))
        ev[f"p1_h1relu_b{b}"] = (A, C.inc(A, 1))
        # PE: P2 block + S2 block
        C.wait(pe, "pe", DMA, ev["ld_w2s"][1])
        C.wait(pe, "pe", A, ev[f"p1_h1relu_b{b}"][1])
        if b - 1 >= 0:
            C.wait(pe, "pe", A, ev[f"p1_s2cp_b{b - 1}"][1])
        pe.append(("p2mm", b))
        ev[f"p1_p2mm_b{b}"] = (P, C.inc(P, 2))
        C.wait(act, "act", P, ev[f"p1_p2mm_b{b}"][1])
        act.append(("p2cp", b))
        ev[f"p1_p2cp_b{b}"] = (A, C.inc(A, 1))
        act.append(("s2cp", b))
        ev[f"p1_s2cp_b{b}"] = (A, C.inc(A, 1))
        C.wait(sp, "sp", A, ev[f"p1_p2cp_b{b}"][1])
        wk = WRS[piece_of_block[b]]
        C.wait(sp, "sp", wk, C.cur(wk))
        sp.append(("p2wr", b))
        ev[f"p1_p2wr_b{b}"] = (wk, C.inc(wk, 16))

    def p2_block_tail(b):
        C.wait(dve, "dve", P, ev[f"p2_aggstop_b{b}"][1])
        C.wait(dve, "dve", V, ev["s2bias"][1])
        if b - 2 >= 0:
            C.wait(dve, "dve", A, ev[f"p2_h2relu_b{b - 2}"][1])
        dve.append(("h2add", b))
        ev[f"p2_h2add_b{b}"] = (V, C.inc(V, 1))
        C.wait(act, "act", V, ev[f"p2_h2add_b{b}"][1])
        act.append(("h2relu", b))
        ev[f"p2_h2relu_b{b}"] = (A, C.inc(A, 1))
        C.wait(sp, "sp", A, ev[f"p2_h2relu_b{b}"][1])
        wk = WRS[piece_of_block[b]]
        C.wait(sp, "sp", wk, C.cur(wk))
        sp.append(("h2wr", b))
        ev[f"p2_h2wr_b{b}"] = (wk, C.inc(wk, 16))

    # ================= PHASE 1 =================
    emit_agg_phase("p1")

    # s2bias after all s2cp
    C.wait(dve, "dve", A, ev[f"p1_s2cp_b{NB - 1}"][1])
    C.wait(dve, "dve", DMA, ev["ld_b2"][1])
    dve.append(("s2bias",))
    ev["s2bias"] = (V, C.inc(V, 1))

    # AG_p2 pieces (pool stream, after p1 gathers)
    for k in range(NPIECES):
        lastb = sum(pn[:k + 1]) - 1
        wk, wv = ev[f"p1_p2wr_b{lastb}"]
        C.wait(pool, "pool", wk, wv)
        pool.append(("ag", "p2", k))
        ev[f"ag_p2_{k}"] = (CC, C.inc(CC, 1))

    # ================= PHASE 2 =================
    emit_agg_phase("p2")

    # AG_h2 pieces
    for k in range(NPIECES):
        lastb = sum(pn[:k + 1]) - 1
        wk, wv = ev[f"p2_h2wr_b{lastb}"]
        C.wait(pool, "pool", wk, wv)
        pool.append(("ag", "h2", k))
        ev[f"ag_h2_{k}"] = (CC, C.inc(CC, 1))

    # ================= PHASE 3 =================
    # windows of <=8 groups within each gather segment
    wseq = [0]
    pending_expand = []

    def p3_emit_window(w, gw0, nb, slot, g0s):
        # pool: one-hot builds (unweighted)
        for g in range(gw0, gw0 + nb):
            C.wait(pool, "pool", DMA, ev["ld_dloc"][1])
            lu = g + OHS * ((Gtot - 1 - g) // OHS)
            C.wait(pool, "pool", P, ev[f"p2_agg_g{lu}"][1])
            if g - OHS >= 0:
                C.wait(pool, "pool", P, ev[f"p3_tr_g{g - OHS}"][1])
            pool.append(("ohu", g))
            ev[f"p3_oh_g{g}"] = (PL, C.inc(PL, 1))
        # PE: transposes into tb bank (w%2)
        for wi, g in enumerate(range(gw0, gw0 + nb)):
            C.wait(pe, "pe", PL, ev[f"p3_oh_g{g}"][1])
            if w - 2 >= 0:
                C.wait(pe, "pe", A, ev[f"p3_ocp_w{w - 2}"][1])
            if w < 2:
                lb = last_user[w % CB]
                C.wait(pe, "pe", V, ev[f"p2_h2add_b{lb}"][1])
            pe.append(("p3tr", g, wi, w % 2))
            ev[f"p3_tr_g{g}"] = (P, C.inc(P, 1))
        ev[f"p3_trdone_w{w}"] = (P, C.cur(P))
        # ACT: batched copy
        C.wait(act, "act", P, ev[f"p3_trdone_w{w}"][1])
        if w - 2 >= 0:
            C.wait(act, "act", P, ev[f"p3_expdone_w{w - 2}"][1])
        act.append(("p3ocp", w, nb))
        ev[f"p3_ocp_w{w}"] = (A, C.inc(A, 1))
        # PE: expands (deferred one window for pipelining)
        pending_expand.append((w, gw0, nb, slot, g0s))
        if len(pending_expand) > 1:
            p3_emit_expand(*pending_expand.pop(0))

    def p3_emit_expand(w, gw0, nb, slot, g0s):
        for wi, g in enumerate(range(gw0, gw0 + nb)):
            C.wait(pe, "pe", A, ev[f"p3_ocp_w{w}"][1])
            if w - 2 >= 0:
                C.wait(pe, "pe", V, ev[f"p3_multdone_w{w - 2}"][1])
            if w < 2:
                lb = last_user[2 + (w % 2)]
                C.wait(pe, "pe", V, ev[f"p2_h2add_b{lb}"][1])
            pe.append(("p3exp", g, wi, w % 2, block_of_g[g]))
            ev[f"p3_exp_g{g}"] = (P, C.inc(P, 1))
        ev[f"p3_expdone_w{w}"] = (P, C.cur(P))
        # DVE: batched mult lo/hi + reduce
        C.wait(dve, "dve", P, ev[f"p3_expdone_w{w}"][1])
        _gs, _gv = ev[f"p3_gather_g{g0s}"]
        C.wait(dve, "dve", _gs, _gv)
        if w - 2 >= 0:
            C.wait(dve, "dve", V, ev[f"p3_reddone_w{w - 2}"][1])
        dve.append(("p3mult", w, gw0, nb, slot, g0s, 0))
        dve.append(("p3mult", w, gw0, nb, slot, g0s, 1))
        mv = C.inc(V, 2)
        ev[f"p3_multdone_w{w}"] = (V, mv)
        for g in range(gw0, gw0 + nb):
            ev[f"p3_multdone_g{g}"] = (V, mv)
        C.wait(dve, "dve", V, ev[f"p3_multdone_w{w}"][1])
        dve.append(("p3red", w, gw0, nb))
        ev[f"p3_reddone_w{w}"] = (V, C.inc(V, 1))

    for (ci, k, g0s, gsum) in gsegs:
        slot = emit_gather("p3", ci, k, g0s, gsum)
        g = g0s
        while g < g0s + gsum:
            nb = min(8, g0s + gsum - g)
            p3_emit_window(wseq[0], g, nb, slot, g0s)
            wseq[0] += 1
            g += nb
    while pending_expand:
        p3_emit_expand(*pending_expand.pop(0))

    # sigmoid + writeback
    lastw = wseq[0] - 1
    C.wait(act, "act", V, ev[f"p3_reddone_w{lastw}"][1])
    act.append(("sigmoid",))
    ev["sig"] = (A, C.inc(A, 1))
    C.wait(sp, "sp", A, ev["sig"][1])
    sp.append(("sxwr",))
    C.inc(DMA, 16)

    # ------------------------------------------------ emit to bass
    from contextlib import ExitStack
    _es = ExitStack()
    with _es:
        idx1_sb = _es.enter_context(nc.sbuf_tensor("idx1_sb", [128, S // 16], I16))
        idx2_sb = _es.enter_context(nc.sbuf_tensor("idx2_sb", [128, S // 16], I16))
        dloc_sb = _es.enter_context(nc.sbuf_tensor("dloc_sb", [128, Gtot], F32))
        val_sb = _es.enter_context(nc.sbuf_tensor("val_sb", [128, Gtot], F32))
        xlT_sb = _es.enter_context(nc.sbuf_tensor("xlT_sb", [128, NPc], BF16))
        w1p_sb = _es.enter_context(nc.sbuf_tensor("w1p_sb", [128, 128], BF16))
        w1s_sb = _es.enter_context(nc.sbuf_tensor("w1s_sb", [128, 128], BF16))
        w2p_sb = _es.enter_context(nc.sbuf_tensor("w2p_sb", [128, 64], BF16))
        w2s_sb = _es.enter_context(nc.sbuf_tensor("w2s_sb", [128, 64], BF16))
        b1_sb = _es.enter_context(nc.sbuf_tensor("b1_sb", [128, 1], F32))
        b2_sb = _es.enter_context(nc.sbuf_tensor("b2_sb", [128, 64], F32))
        iota_sb = _es.enter_context(nc.sbuf_tensor("iota_sb", [128, 128], BF16))
        ident_sb = _es.enter_context(nc.sbuf_tensor("ident_sb", [128, 128], BF16))
        oh_sb = _es.enter_context(nc.sbuf_tensor("oh_sb", [128, OHS, 128], BF16))
        gbuf = _es.enter_context(
            nc.sbuf_tensor("gbuf", [128, NSLOT, GH * 128], BF16))
        h1T_sb = _es.enter_context(nc.sbuf_tensor("h1T_sb", [128, NPc], BF16))
        aggT_sb = _es.enter_context(nc.sbuf_tensor("aggT_sb", [128, 2, 128], BF16))
        s2_sb = _es.enter_context(nc.sbuf_tensor("s2_sb", [128, NB, 64], F32))
        h2nm_sb = _es.enter_context(nc.sbuf_tensor("h2nm_sb", [128, NB, 64], BF16))
        p2nm_sb = _es.enter_context(nc.sbuf_tensor("p2nm_sb", [128, NB, 64], BF16))
        h2pre_sb = _es.enter_context(nc.sbuf_tensor("h2pre_sb", [128, 2, 64], F32))
        osb_sb = _es.enter_context(nc.sbuf_tensor("osb_sb", [128, 2, 8 * 128], BF16))
        prod_sb = _es.enter_context(nc.sbuf_tensor("prod_sb", [128, 2, 8, 64], F32))
        dots_sb = _es.enter_context(nc.sbuf_tensor("dots_sb", [128, Gtot], F32))
        aggb = [_es.enter_context(nc.psum_tensor(f"aggb{j}", [128, 512], F32))
                for j in range(CB)]
        h1b = _es.enter_context(nc.psum_tensor("h1b", [128, 512], F32))
        p2s2b = _es.enter_context(nc.psum_tensor("p2s2b", [128, 512], F32))
        dma_s = _es.enter_context(nc.semaphore("dma_s"))
        gt_sems = [_es.enter_context(nc.semaphore(f"gt{j}_s"))
                   for j in range(NSLOT)]
        wr_sems = [_es.enter_context(nc.semaphore(f"wr{j}_s"))
                   for j in range(NPIECES)]
        v_s = _es.enter_context(nc.semaphore("v_s"))
        a_s = _es.enter_context(nc.semaphore("a_s"))
        p_s = _es.enter_context(nc.semaphore("p_s"))
        pl_s = _es.enter_context(nc.semaphore("pl_s"))
        cc_s = _es.enter_context(nc.semaphore("cc_s"))
        block = _es.enter_context(nc.Block())
        sems = {DMA: dma_s, V: v_s, A: a_s, P: p_s, PL: pl_s, CC: cc_s}
        for j in range(NSLOT):
            sems[GTS[j]] = gt_sems[j]
        for j in range(NPIECES):
            sems[WRS[j]] = wr_sems[j]

        sb_map = {"idx1": idx1_sb, "idx2": idx2_sb, "dloc": dloc_sb,
                  "val": val_sb, "xlT": xlT_sb, "w1p": w1p_sb, "w1s": w1s_sb,
                  "w2p": w2p_sb, "w2s": w2s_sb, "b1": b1_sb, "b2": b2_sb}
        in_map = {"idx1": idx1_in, "idx2": idx2_in, "dloc": dloc_in,
                  "val": val_in, "xlT": xlT_in, "w1p": w1p_in, "w1s": w1s_in,
                  "w2p": w2p_in, "w2s": w2s_in, "b1": b1_in, "b2": b2_in}

        def gv(slot, j):
            return gbuf[:, slot, j * 128:(j + 1) * 128]

        def run_ops(eng, name):
            for op in ops[name]:
                kind = op[0]
                if kind == "wait":
                    eng.wait_ge(sems[op[1]], op[2])
                elif kind == "dma_sb":
                    eng.dma_start(out=sb_map[op[1]][:], in_=in_map[op[1]][:]
                                  ).then_inc(dma_s, 16)
                elif kind == "iota":
                    eng.iota(iota_sb[:], pattern=[[1, 128]], base=0,
                             channel_multiplier=0,
                             allow_small_or_imprecise_dtypes=True)
                    eng.drain()
                elif kind == "ident":
                    eng.memset(ident_sb[:], 0.0)
                    eng.drain()
                    masks.make_identity(nc, ident_sb[:], nomemset=True)
                    eng.drain()
                    eng.memset(ident_sb[:1, :1], 1.0).then_inc(pl_s, 1)
                elif kind == "gather":
                    _, ph, k, g0, gsum, slot = op
                    if ph == "p1":
                        tbl = xg[int(trowbase[k]):int(trowbase[k + 1]), :]
                        idxs = idx1_sb
                    else:
                        t = p2t if ph == "p2" else h2t
                        tbl = t[pairbase[k]:pairbase[k + 1], :]
                        idxs = idx2_sb
                    eng.dma_gather(
                        gbuf[:, slot, :gsum * 128].rearrange(
                            "p (g f) -> p g f", f=128),
                        tbl,
                        idxs[:, g0 * 8:(g0 + gsum) * 8],
                        num_idxs=gsum * 128, num_idxs_reg=gsum * 128,
                        elem_size=128, single_packet=False,
                    ).then_inc(gt_sems[slot], 16)
                elif kind == "ohw":
                    g = op[1]
                    eng.tensor_scalar(out=oh_sb[:, g % OHS, :],
                                      in0=iota_sb[:],
                                      scalar1=dloc_sb[:, g:g + 1],
                                      scalar2=val_sb[:, g:g + 1],
                                      op0=ALU.is_equal,
                                      op1=ALU.mult).then_inc(v_s, 1)
                elif kind == "ohu":
                    g = op[1]
                    eng.tensor_scalar(out=oh_sb[:, g % OHS, :],
                                      in0=iota_sb[:],
                                      scalar1=dloc_sb[:, g:g + 1],
                                      scalar2=None,
                                      op0=ALU.is_equal).then_inc(pl_s, 1)
                elif kind == "agg1":
                    _, b, g, j, slot, first, last = op
                    eng.matmul(aggb[b % CB][:, :128], lhsT=gv(slot, j),
                               rhs=oh_sb[:, g % OHS, :], start=first,
                               stop=last).then_inc(p_s, 1)
                elif kind == "agg2":
                    _, b, g, j, slot, first, last = op
                    gvj = gv(slot, j)
                    eng.matmul(aggb[b % CB][:, :64],
                               lhsT=oh_sb[:64, g % OHS, :],
                               rhs=gvj[:64, :64], start=first,
                               stop=False).then_inc(p_s, 1)
                    eng.matmul(aggb[b % CB][:, :64],
                               lhsT=oh_sb[64:, g % OHS, :],
                               rhs=gvj[64:, 64:], start=False,
                               stop=last).then_inc(p_s, 1)
                elif kind == "aggcopy":
                    b = op[1]
                    eng.activation(aggT_sb[:, b % 2, :], aggb[b % CB][:, :128],
                                   AF.Copy).then_inc(a_s, 1)
                elif kind == "h1mm":
                    b = op[1]
                    eng.matmul(h1b[:, :128], lhsT=w1p_sb[:],
                               rhs=aggT_sb[:, b % 2, :], start=True,
                               stop=False).then_inc(p_s, 1)
                    eng.matmul(h1b[:, :128], lhsT=w1s_sb[:],
                               rhs=xlT_sb[:, b * 128:(b + 1) * 128],
                               start=False, stop=True).then_inc(p_s, 1)
                elif kind == "h1relu":
                    b = op[1]
                    eng.activation(h1T_sb[:, b * 128:(b + 1) * 128],
                                   h1b[:, :128], AF.Relu, bias=b1_sb[:]
                                   ).then_inc(a_s, 1)
                elif kind == "p2mm":
                    b = op[1]
                    eng.matmul(p2s2b[:, :64],
                               lhsT=h1T_sb[:, b * 128:(b + 1) * 128],
                               rhs=w2p_sb[:], start=True, stop=True
                               ).then_inc(p_s, 1)
                    eng.matmul(p2s2b[:, 64:128],
                               lhsT=h1T_sb[:, b * 128:(b + 1) * 128],
                               rhs=w2s_sb[:], start=True, stop=True
                               ).then_inc(p_s, 1)
                elif kind == "p2cp":
                    b = op[1]
                    eng.activation(p2nm_sb[:, b, :], p2s2b[:, :64],
                                   AF.Copy).then_inc(a_s, 1)
                elif kind == "s2cp":
                    b = op[1]
                    eng.activation(s2_sb[:, b, :], p2s2b[:, 64:128],
                                   AF.Copy).then_inc(a_s, 1)
                elif kind == "p2wr":
                    b = op[1]
                    eng.dma_start(out=p2_loc[b * 128:(b + 1) * 128, :],
                                  in_=p2nm_sb[:, b, :]).then_inc(
                        wr_sems[piece_of_block[b]], 16)
                elif kind == "s2bias":
                    eng.tensor_tensor(
                        out=s2_sb[:], in0=s2_sb[:],
                        in1=b2_sb[:, None, :].to_broadcast([128, NB, 64]),
                        op=ALU.add).then_inc(v_s, 1)
                elif kind == "ag":
                    _, which, k = op
                    loc = p2_loc if which == "p2" else h2_loc
                    tab = p2t if which == "p2" else h2t
                    eng.collective_compute(
                        "AllGather", ALU.bypass,
                        replica_groups=[list(range(NCORES))],
                        ins=[loc[locbase[k]:locbase[k] + pn[k] * 128, :]],
                        outs=[tab[pairbase[k]:pairbase[k + 1], :]],
                    ).then_inc(cc_s, 1)
                elif kind == "h2add":
                    b = op[1]
                    eng.tensor_tensor(out=h2pre_sb[:, b % 2, :],
                                      in0=aggb[b % CB][:, :64],
                                      in1=s2_sb[:, b, :],
                                      op=ALU.add).then_inc(v_s, 1)
                elif kind == "h2relu":
                    b = op[1]
                    eng.activation(h2nm_sb[:, b, :], h2pre_sb[:, b % 2, :],
                                   AF.Relu).then_inc(a_s, 1)
                elif kind == "h2wr":
                    b = op[1]
                    eng.dma_start(out=h2_loc[b * 128:(b + 1) * 128, :],
                                  in_=h2nm_sb[:, b, :]).then_inc(
                        wr_sems[piece_of_block[b]], 16)
                elif kind == "p3tr":
                    _, g, wi, tb = op
                    eng.transpose(
                        out=aggb[tb][:].bitcast(BF16)[:, wi * 128:(wi + 1) * 128],
                        in_=oh_sb[:, g % OHS, :],
                        identity=ident_sb[:]).then_inc(p_s, 1)
                elif kind == "p3ocp":
                    _, w, nb = op
                    eng.activation(
                        osb_sb[:, w % 2, :nb * 128],
                        aggb[w % 2][:].bitcast(BF16)[:, :nb * 128],
                        AF.Copy).then_inc(a_s, 1)
                elif kind == "p3exp":
                    _, g, wi, eb, b = op
                    eng.matmul(aggb[2 + eb][:, wi * 64:(wi + 1) * 64],
                               lhsT=osb_sb[:, eb, wi * 128:(wi + 1) * 128],
                               rhs=h2nm_sb[:, b, :], start=True, stop=True
                               ).then_inc(p_s, 1)
                elif kind == "p3mult":
                    _, w, gw0, nb, slot, g0s, hi = op
                    pr = slice(64, 128) if hi else slice(0, 64)
                    cr = slice(64, 128) if hi else slice(0, 64)
                    j0 = gw0 - g0s
                    eng.tensor_tensor(
                        out=prod_sb[pr, w % 2, :nb, :],
                        in0=aggb[2 + w % 2][pr, :nb * 64].rearrange(
                            "p (g f) -> p g f", f=64),
                        in1=gbuf[pr, slot, j0 * 128:(j0 + nb) * 128].rearrange(
                            "p (g f) -> p g f", f=128)[:, :, cr],
                        op=ALU.mult).then_inc(v_s, 1)
                elif kind == "p3red":
                    _, w, gw0, nb = op
                    eng.reduce_sum(out=dots_sb[:, gw0:gw0 + nb],
                                   in_=prod_sb[:, w % 2, :nb, :],
                                   axis=mybir.AxisListType.X).then_inc(v_s, 1)
                elif kind == "sigmoid":
                    eng.activation(dots_sb[:], dots_sb[:], AF.Sigmoid
                                   ).then_inc(a_s, 1)
                elif kind == "sxwr":
                    eng.dma_start(out=sx_out[:], in_=dots_sb[:]
                                  ).then_inc(dma_s, 16)
                else:
                    raise ValueError(kind)

        @block.sync
        def _(e):
            run_ops(e, "sp")

        @block.gpsimd
        def _(e):
            run_ops(e, "pool")

        @block.vector
        def _(e):
            run_ops(e, "dve")

        @block.scalar
        def _(e):
            run_ops(e, "act")

        @block.tensor
        def _(e):
            run_ops(e, "pe")

    nc.compile()
    return nc


# ---------------------------------------------------------------- host glue
def host_prep(X, edge_row, edge_col, edge_vals, W1p, b1p, W1s, b1s,
              W2p, b2p, W2s, b2s, plan):
    p = plan
    NP, NPc = p.NP, p.NPc
    Xp = np.zeros((NP, X.shape[1]), np.float32)
    Xp[: X.shape[0]] = X
    Xperm = Xp[p.perm]                       # (c,b,l)-ordered
    Xtab = Xp[p.node_of_trow]                # table-row ordered
    Xg = Xtab.astype(ml_dtypes.bfloat16)
    b1 = np.ascontiguousarray((b1p + b1s).astype(np.float32)[:, None])
    b2rep = np.ascontiguousarray(
        np.tile((b2p + b2s).astype(np.float32)[None, :], (128, 1)))
    in_maps = []
    for c in range(NCORES):
        in_maps.append({
            "xg": Xg,
            "xlT": np.ascontiguousarray(
                Xperm[c * NPc:(c + 1) * NPc].T).astype(ml_dtypes.bfloat16),
            "idx1": wrap_idx(p.idx1[c]),
            "idx2": wrap_idx(p.idx2[c]),
            "dloc": colmajor(p.dloc[c]),
            "val": colmajor(p.val[c]),
            "w1p": np.ascontiguousarray(W1p).astype(ml_dtypes.bfloat16),
            "w1s": np.ascontiguousarray(W1s).astype(ml_dtypes.bfloat16),
            "w2p": np.ascontiguousarray(W2p).astype(ml_dtypes.bfloat16),
            "w2s": np.ascontiguousarray(W2s).astype(ml_dtypes.bfloat16),
            "b1": b1, "b2rep": b2rep,
        })
    return in_maps


def unpermute_sx(results, plan, n_edges):
    p = plan
    sx = np.empty(n_edges, np.float32)
    for c in range(NCORES):
        flat = results[c]["sx"].T.reshape(-1)
        m = p.core_of_edge[:n_edges] == c
        sx[m] = flat[p.slot_of_edge[m]]
    return sx


_CACHE = {}


def kernel(X, edge_row, edge_col, edge_vals,
           W_pass1, b_pass1, W_self1, b_self1,
           W_pass2, b_pass2, W_self2, b_self2):
    X = np.asarray(X, np.float32)
    er = np.asarray(edge_row).astype(np.int64)
    ec = np.asarray(edge_col).astype(np.int64)
    ev_ = np.asarray(edge_vals, np.float32)
    n_nodes, n_edges = X.shape[0], len(er)

    key = (n_nodes, n_edges, int(er[0]), int(ec[0]))
    if key not in _CACHE:
        plan = plan_graph(er, ec, ev_, n_nodes)
        nc = build(plan)
        _CACHE[key] = (plan, nc)
    plan, nc = _CACHE[key]

    in_maps = host_prep(X, er, ec, ev_,
                        np.asarray(W_pass1), np.asarray(b_pass1),
                        np.asarray(W_self1), np.asarray(b_self1),
                        np.asarray(W_pass2), np.asarray(b_pass2),
                        np.asarray(W_self2), np.asarray(b_self2), plan)
    res = run_bass_kernel_spmd(nc, in_maps, core_ids=list(range(NCORES)))
    return unpermute_sx(res.results, plan, n_edges)
